# revision 56
# baseline (speedup 1.0000x reference)
"""Trainium2 Bass kernel for nn_KalmanGraphicalModel (gnn_message_passing).

The reference runs ITERS=100 iterations of a LINEAR 3-point stencil in time:
    x <- A' x_t + B' x_{t-1} + C' x_{t+1} + Gam y_t     (edge-replicated)
The composed 100-step operator is a banded convolution with tiny bandwidth
D (<=8 at ~2e-5 relative truncation for gamma=0.01):
    x_100[t] = sum_{|d|<=D} G_d x0[t+d] + V_d y[t+d]
One banded-matmul pass on device, 3 column-passes per 512-col PSUM tile:
  - time axis folded 16-way into the partition dim (16 blocks x 8 rows = 128)
  - block-band sigma in {-1,0,+1}; with D<=8 the sigma=-1 block matrix only
    has nonzero contraction rows in fold-blocks 8..15 (partitions 64..127)
    and sigma=+1 only in fold-blocks 0..7 (partitions 0..63), so BOTH outer
    x taps pack into ONE matmul against a half-shifted x copy
    xpk = [x[0:64] shift2; x[64:128] shift0].  The outer-tap mass is ~2%
    of the operator, so that stream rides in fp8 (halving its HBM bytes;
    measured cost ~5e-3 rel err, gate is 2e-2).
  - y: center tap (64 rows) + the two outer blocks (32 disjoint rows each)
    pack into ONE 128-contraction matmul against
    yq = [y shift1; y[0:32] shift2; y[32:64] shift0].
  - everything else bf16 in and out (output upcast on host)
  - DMAs on 3 queues (sync=wb+x, scalar=yq, gpsimd=w8+xpk+some outs) in
    per-tile 512-col chunks; a DMA only completes when the slowest of its
    16 stripe engines finishes, so the input stream paces the chain
  - warmup matmuls over a memset scratch tile keep the tensor engine busy
    through the input load so its clock (p-state) has ramped 2x by the
    time the real chain starts
T is sharded across 8 cores; the first/last 128 columns (edge-rule
influenced + window zero-padding) are computed host-side on tiny strips.
"""
import os
import numpy as np

N, M, T, ITERS = 8, 4, 500000, 100
NCORES = 8
L = T // NCORES          # 62500 timesteps per core
FOLD = 16                # 16 blocks x 8 rows = 128 partitions
NC = 3908                # out cols per core: 16*3908 = 62528 >= 62500
CW = NC + 2              # input window cols (1-col halo each side)
EDGE = 128               # host-computed override width at the two true edges
STRIP = 384              # width of host edge strips
TS = 512                 # PSUM tile cols
DMAX = 8                 # tap truncation: |d|<=8 keeps the outer blocks in
                         # disjoint partition halves (tap d=9 is ~2e-6 rel)

_PROGRAM_CACHE = {}
WARM = int(os.environ.get("KALMAN_WARM", "12"))      # PE p-state warmup mms
F8PK = bool(int(os.environ.get("KALMAN_F8PK", "1")))  # fp8 outer-x stream
F8S = float(os.environ.get("KALMAN_F8S", "16"))       # fp8 weight scale
F8Y = bool(int(os.environ.get("KALMAN_F8Y", "0")))    # fp8 y stream too
DSCR = int(os.environ.get("KALMAN_DSCR", "16384"))    # dynamic DGE scratch


def _compose_taps(F, H, Q, R, gamma):
    """Banded composition of the 100 linear steps, in float64."""
    Qinv = np.linalg.inv(Q)
    Rinv = np.linalg.inv(R)
    negQinv = -Qinv
    FtQinv = F.T @ Qinv
    HtRinv = H.T @ Rinv
    Z1 = np.eye(N); Z1[0, 0] = 0.0
    Z2 = np.eye(N); Z2[-1, -1] = 0.0
    Ap = np.eye(N) + gamma * (negQinv @ Z1 - FtQinv @ Z2 @ F - HtRinv @ H)
    Bp = -gamma * (negQinv @ Z1 @ F)
    Cp = gamma * (FtQinv @ Z2)
    Gam = gamma * HtRinv

    K = ITERS
    G = np.zeros((2 * K + 1, N, N))
    V = np.zeros((2 * K + 1, N, M))
    G[K] = np.eye(N)
    for _ in range(K):
        Gn = np.einsum("ij,djk->dik", Ap, G)
        Gn[:-1] += np.einsum("ij,djk->dik", Bp, G[1:])
        Gn[1:] += np.einsum("ij,djk->dik", Cp, G[:-1])
        Vn = np.einsum("ij,djk->dik", Ap, V)
        Vn[:-1] += np.einsum("ij,djk->dik", Bp, V[1:])
        Vn[1:] += np.einsum("ij,djk->dik", Cp, V[:-1])
        Vn[K] += Gam
        G, V = Gn, Vn
    return G, V, (Ap.astype(np.float32), Bp.astype(np.float32),
                  Cp.astype(np.float32), Gam.astype(np.float32))


def _build_program():
    import concourse.tile as tile
    from concourse import bacc, mybir

    key = ("v18", WARM, F8PK, F8Y, DSCR)
    if key in _PROGRAM_CACHE:
        return _PROGRAM_CACHE[key]

    f32 = mybir.dt.float32
    bf16 = mybir.dt.bfloat16
    f8 = mybir.dt.float8e4
    xpk_dt = f8 if F8PK else bf16

    ndev = int(os.environ.get("KALMAN_NDEV", "1"))
    nc = bacc.Bacc("TRN2", target_bir_lowering=False, debug=False,
                   enable_asserts=False, num_devices=ndev,
                   dynamic_dma_scratch_size=DSCR)
    xb = nc.dram_tensor("xb", [128, CW], bf16, kind="ExternalInput").ap()
    xpk = nc.dram_tensor("xpk", [128, CW], xpk_dt, kind="ExternalInput").ap()
    yq_dt = f8 if F8Y else bf16
    yq = nc.dram_tensor("yq", [128, CW], yq_dt, kind="ExternalInput").ap()
    wb = nc.dram_tensor("wb", [128, 384], bf16, kind="ExternalInput").ap()
    if F8PK:
        w8 = nc.dram_tensor("w8", [128, 128], f8, kind="ExternalInput").ap()
    if F8Y:
        w8y = nc.dram_tensor("w8y", [128, 128], f8, kind="ExternalInput").ap()
    out = nc.dram_tensor("out", [128, NC], bf16, kind="ExternalOutput").ap()

    tiles = []
    c = 0
    while c < NC:
        tiles.append((c, min(TS, NC - c)))
        c += TS

    # per-tile input chunks; tile k touches cols [k*TS, (k+1)*TS+2), so the
    # first chunk is TS+2 wide and the rest shift by TS: tile k then depends
    # on chunks 0..k only.
    bounds = [0, TS + 2]
    while bounds[-1] + TS < CW:
        bounds.append(bounds[-1] + TS)
    bounds.append(CW)
    chunks = [(bounds[i], bounds[i + 1] - bounds[i])
              for i in range(len(bounds) - 1)]
    # the fp8 stream is half the bytes: pair up its chunks
    xpk_chunks = []
    for k in range(0, len(chunks), 2):
        pc0 = chunks[k][0]
        pcn = (chunks[k][1] + chunks[k + 1][1]
               if k + 1 < len(chunks) else chunks[k][1])
        xpk_chunks.append((pc0, pcn))

    with tile.TileContext(nc) as tc:
        with tc.tile_pool(name="consts", bufs=1) as consts, \
             tc.tile_pool(name="psw", bufs=1, space="PSUM") as psw_pool, \
             tc.tile_pool(name="ps", bufs=7, space="PSUM") as ps_pool:
            wbsb = consts.tile([128, 384], bf16)
            xsb = consts.tile([128, CW], bf16)
            xpsb = consts.tile([128, CW], xpk_dt)
            ysb = consts.tile([128, CW], yq_dt)
            osb = consts.tile([128, NC], bf16)
            if F8PK:
                w8sb = consts.tile([128, 128], f8)
                wosb = w8sb[:]
            else:
                wosb = wbsb[:, 128:256]
            if F8Y:
                w8ysb = consts.tile([128, 128], f8)
                wysb = w8ysb[:]
            else:
                wysb = wbsb[:, 256:384]
            scr = consts.tile([128, 512], bf16)

            # PE p-state warmup: matmuls over a memset scratch tile (no DMA
            # dependency) keep the tensor engine busy through the input load
            # so the clock has ramped when the real chain starts.
            nc.gpsimd.memset(scr[:], 0.0)
            if WARM:
                psw = psw_pool.tile([128, 512], f32)
                for _ in range(WARM):
                    nc.tensor.matmul(psw[:], scr[:, 0:128], scr[:],
                                     start=True, stop=True)

            # queue layout: sync = wb + x chunks, scalar = y chunks,
            # gpsimd (software DGE, slower) = the small fp8 stream
            nc.sync.dma_start(wbsb[:], wb[:])
            if F8PK:
                nc.gpsimd.dma_start(w8sb[:], w8[:])
            if F8Y:
                nc.gpsimd.dma_start(w8ysb[:], w8y[:])
            for (c0, cn) in chunks:
                nc.sync.dma_start(xsb[:, c0:c0 + cn], xb[:, c0:c0 + cn])
                nc.scalar.dma_start(ysb[:, c0:c0 + cn], yq[:, c0:c0 + cn])
            for (pc0, pcn) in xpk_chunks:
                nc.gpsimd.dma_start(xpsb[:, pc0:pc0 + pcn],
                                    xpk[:, pc0:pc0 + pcn])

            ndone = 0
            for ti, (c0, cn) in enumerate(tiles):
                ps = ps_pool.tile([128, cn], f32)
                # center x tap (sigma=0): moving offset c0+1
                nc.tensor.matmul(ps[:], wbsb[:, 0:128],
                                 xsb[:, c0 + 1:c0 + 1 + cn],
                                 start=True, stop=False)
                # both outer x taps in one pass against half-shifted x
                nc.tensor.matmul(ps[:], wosb,
                                 xpsb[:, c0:c0 + cn],
                                 start=False, stop=False)
                # all three y taps in one pass
                nc.tensor.matmul(ps[:], wysb,
                                 ysb[:, c0:c0 + cn],
                                 start=False, stop=True)
                nc.vector.tensor_copy(osb[:, c0:c0 + cn], ps[:])
                # drain finished output columns after odd tiles and the
                # last three, spread across queues so no queue's backlog
                # delays the final drain (sync's x stream ends earliest,
                # so it takes the last, smallest chunk)
                if ti % 2 == 1 or ti >= len(tiles) - 2:
                    o0, o1 = ndone, c0 + cn
                    ndone = o1
                    eng = {1: nc.gpsimd, 3: nc.scalar, 5: nc.gpsimd,
                           6: nc.scalar, 7: nc.sync}[ti]
                    eng.dma_start(out[:, o0:o1], osb[:, o0:o1])
    nc.compile()
    _PROGRAM_CACHE[key] = nc
    return nc


def _fold(a, rows, width):
    # a: (rows, 16*width) -> (rows*16 partitions, width); partition b*rows+r
    # holds times t = c*16 + b
    return np.ascontiguousarray(
        a.reshape(rows, width, FOLD).transpose(2, 0, 1).reshape(
            FOLD * rows, width))


def _run_edge_strip(x0, y, Ap, Bp, Cp, Gam):
    x = x0.copy()
    for _ in range(ITERS):
        xp = np.concatenate([x[:, :1], x[:, :-1]], axis=1)
        xf_ = np.concatenate([x[:, 1:], x[:, -1:]], axis=1)
        x = (Ap @ x + Bp @ xp + Cp @ xf_ + Gam @ y).astype(np.float32)
    return x


def kernel(xs, ys, F, H, Q, R, gamma):
    import ml_dtypes
    from concourse.bass_utils import run_bass_kernel_spmd

    bf16 = np.dtype(ml_dtypes.bfloat16)

    xs = np.asarray(xs, dtype=np.float32)
    ysv = np.asarray(ys, dtype=np.float32)
    g = float(np.asarray(gamma))

    G, V, mats32 = _compose_taps(
        np.asarray(F, np.float64), np.asarray(H, np.float64),
        np.asarray(Q, np.float64), np.asarray(R, np.float64), g)
    K = ITERS
    D = DMAX
    # sanity: dropped taps must be tiny relative to the kept mass
    drop = max(np.abs(G[K + D + 1:K + 2 * D]).max(initial=0),
               np.abs(G[K - 2 * D:K - D]).max(initial=0))
    assert drop < 1e-4 * np.abs(G).max(), f"tap truncation too lossy: {drop}"

    # ---- block-banded weights, sigma in {-1,0,+1} == si in {0,1,2} ----
    WX = np.zeros((3, 128, 128), dtype=np.float32)
    WY = np.zeros((3, 64, 128), dtype=np.float32)
    for si in range(3):
        sig = si - 1
        for bo in range(FOLD):
            for bi in range(FOLD):
                d = sig * FOLD + bi - bo
                if abs(d) > D:
                    continue
                WX[si, bi * 8:bi * 8 + 8, bo * 8:bo * 8 + 8] = G[K + d].T
                WY[si, bi * 4:bi * 4 + 4, bo * 8:bo * 8 + 8] = V[K + d].T
    # D<=8 guarantees the outer blocks live in disjoint partition halves
    assert not WX[0][:64].any() and not WX[2][64:].any()
    assert not WY[0][:32].any() and not WY[2][32:].any()

    wb_np = np.zeros((128, 384), dtype=np.float32)
    wb_np[:, 0:128] = WX[1]
    # packed outer-x stationary: rows 0:64 pair with x shift +2 (sigma=+1),
    # rows 64:128 with x shift 0 (sigma=-1)
    wo_np = np.zeros((128, 128), dtype=np.float32)
    wo_np[0:64] = WX[2][:64]
    wo_np[64:128] = WX[0][64:]
    wb_np[:, 128:256] = wo_np
    # packed y stationary: rows 0:64 = center tap (y shift 1), rows 64:96 =
    # sigma=+1 block rows (y[0:32] shift 2), rows 96:128 = sigma=-1 block
    # rows (y[32:64] shift 0)
    wy_np = np.zeros((128, 128), dtype=np.float32)
    wy_np[0:64] = WY[1]
    wy_np[64:96] = WY[2][:32]
    wy_np[96:128] = WY[0][32:]
    wb_np[:, 256:384] = wy_np
    wb_np = wb_np.astype(bf16)

    # ---- per-core folded input windows ----
    pad = FOLD                               # S=1 halo in timesteps
    xw = FOLD * (CW + 2)
    xs_p = np.zeros((N, 7 * L + xw), dtype=np.float32)
    ys_p = np.zeros((M, 7 * L + xw), dtype=np.float32)
    xs_p[:, pad:pad + T] = xs
    ys_p[:, pad:pad + T] = ysv
    f8np = np.dtype(ml_dtypes.float8_e4m3)

    def to_f8(a):
        return a.astype(f8np)
    if F8PK:
        w8_np = to_f8(wo_np * F8S)
    if F8Y:
        w8y_np = to_f8(wy_np * F8S)
    in_maps = []
    for i in range(NCORES):
        o = i * L
        xf = _fold(xs_p[:, o:o + xw], N, CW + 2)
        yf = _fold(ys_p[:, o:o + xw], M, CW + 2)
        xpk_np = np.concatenate([xf[0:64, 2:CW + 2], xf[64:128, 0:CW]],
                                axis=0)
        yq_np = np.concatenate([yf[:, 1:CW + 1], yf[0:32, 2:CW + 2],
                                yf[32:64, 0:CW]], axis=0)
        m_ = {
            "xb": np.ascontiguousarray(xf[:, 0:CW]).astype(bf16),
            "xpk": to_f8(xpk_np / F8S) if F8PK else xpk_np.astype(bf16),
            "yq": to_f8(yq_np / F8S) if F8Y else yq_np.astype(bf16),
            "wb": wb_np,
        }
        if F8PK:
            m_["w8"] = w8_np
        if F8Y:
            m_["w8y"] = w8y_np
        in_maps.append(m_)

    nc = _build_program()
    trace = bool(int(os.environ.get("KALMAN_TRACE", "0")))
    res = run_bass_kernel_spmd(nc, in_maps, core_ids=list(range(NCORES)),
                               trace=trace)
    if trace and res.exec_time_ns is not None:
        print(f"HW exec time: {res.exec_time_ns} ns")
        print(f"HW exec time mean: {res.mean_exec_time_ns} ns")

    out_full = np.empty((N, T), dtype=np.float32)
    for i in range(NCORES):
        o = i * L
        Out = np.asarray(res.results[i]["out"]).astype(np.float32)  # (128,NC)
        unf = Out.reshape(FOLD, N, NC).transpose(1, 2, 0).reshape(N, FOLD * NC)
        out_full[:, o:o + L] = unf[:, :L]

    # ---- host edge strips (exact edge-replication dynamics) ----
    Ap32, Bp32, Cp32, Gam32 = mats32
    left = _run_edge_strip(xs[:, :STRIP], ysv[:, :STRIP],
                           Ap32, Bp32, Cp32, Gam32)
    right = _run_edge_strip(xs[:, -STRIP:], ysv[:, -STRIP:],
                            Ap32, Bp32, Cp32, Gam32)
    out_full[:, :EDGE] = left[:, :EDGE]
    out_full[:, -EDGE:] = right[:, -EDGE:]
    return out_full


# revision 57
# speedup vs baseline: 1.0228x; 1.0228x over previous
"""Trainium2 Bass kernel for nn_KalmanGraphicalModel (gnn_message_passing).

The reference runs ITERS=100 iterations of a LINEAR 3-point stencil in time:
    x <- A' x_t + B' x_{t-1} + C' x_{t+1} + Gam y_t     (edge-replicated)
The composed 100-step operator is a banded convolution with tiny bandwidth
D (<=8 at ~2e-5 relative truncation for gamma=0.01):
    x_100[t] = sum_{|d|<=D} G_d x0[t+d] + V_d y[t+d]
One banded-matmul pass on device, 3 column-passes per 512-col PSUM tile:
  - time axis folded 16-way into the partition dim (16 blocks x 8 rows = 128)
  - block-band sigma in {-1,0,+1}; with D<=8 the sigma=-1 block matrix only
    has nonzero contraction rows in fold-blocks 8..15 (partitions 64..127)
    and sigma=+1 only in fold-blocks 0..7 (partitions 0..63), so BOTH outer
    x taps pack into ONE matmul against a half-shifted x copy
    xpk = [x[0:64] shift2; x[64:128] shift0].  The outer-tap mass is ~2%
    of the operator, so that stream rides in fp8 (halving its HBM bytes;
    measured cost ~5e-3 rel err, gate is 2e-2).
  - y: center tap (64 rows) + the two outer blocks (32 disjoint rows each)
    pack into ONE 128-contraction matmul against
    yq = [y shift1; y[0:32] shift2; y[32:64] shift0].
  - everything else bf16 in and out (output upcast on host)
  - DMAs on 3 queues (sync=wb+x, scalar=yq, gpsimd=w8+xpk+some outs) in
    per-tile 512-col chunks; a DMA only completes when the slowest of its
    16 stripe engines finishes, so the input stream paces the chain
  - warmup matmuls over a memset scratch tile keep the tensor engine busy
    through the input load so its clock (p-state) has ramped 2x by the
    time the real chain starts
T is sharded across 8 cores; the first/last 128 columns (edge-rule
influenced + window zero-padding) are computed host-side on tiny strips.
"""
import os
import numpy as np

N, M, T, ITERS = 8, 4, 500000, 100
NCORES = 8
L = T // NCORES          # 62500 timesteps per core
FOLD = 16                # 16 blocks x 8 rows = 128 partitions
NC = 3908                # out cols per core: 16*3908 = 62528 >= 62500
CW = NC + 2              # input window cols (1-col halo each side)
EDGE = 128               # host-computed override width at the two true edges
STRIP = 384              # width of host edge strips
TS = 512                 # PSUM tile cols
DMAX = 8                 # tap truncation: |d|<=8 keeps the outer blocks in
                         # disjoint partition halves (tap d=9 is ~2e-6 rel)

_PROGRAM_CACHE = {}
WARM = int(os.environ.get("KALMAN_WARM", "12"))      # PE p-state warmup mms
F8PK = bool(int(os.environ.get("KALMAN_F8PK", "1")))  # fp8 outer-x stream
F8S = float(os.environ.get("KALMAN_F8S", "16"))       # fp8 weight scale
F8Y = bool(int(os.environ.get("KALMAN_F8Y", "0")))    # fp8 y stream too
DSCR = int(os.environ.get("KALMAN_DSCR", "16384"))    # dynamic DGE scratch


def _compose_taps(F, H, Q, R, gamma):
    """Banded composition of the 100 linear steps, in float64."""
    Qinv = np.linalg.inv(Q)
    Rinv = np.linalg.inv(R)
    negQinv = -Qinv
    FtQinv = F.T @ Qinv
    HtRinv = H.T @ Rinv
    Z1 = np.eye(N); Z1[0, 0] = 0.0
    Z2 = np.eye(N); Z2[-1, -1] = 0.0
    Ap = np.eye(N) + gamma * (negQinv @ Z1 - FtQinv @ Z2 @ F - HtRinv @ H)
    Bp = -gamma * (negQinv @ Z1 @ F)
    Cp = gamma * (FtQinv @ Z2)
    Gam = gamma * HtRinv

    K = ITERS
    G = np.zeros((2 * K + 1, N, N))
    V = np.zeros((2 * K + 1, N, M))
    G[K] = np.eye(N)
    for _ in range(K):
        Gn = np.einsum("ij,djk->dik", Ap, G)
        Gn[:-1] += np.einsum("ij,djk->dik", Bp, G[1:])
        Gn[1:] += np.einsum("ij,djk->dik", Cp, G[:-1])
        Vn = np.einsum("ij,djk->dik", Ap, V)
        Vn[:-1] += np.einsum("ij,djk->dik", Bp, V[1:])
        Vn[1:] += np.einsum("ij,djk->dik", Cp, V[:-1])
        Vn[K] += Gam
        G, V = Gn, Vn
    return G, V, (Ap.astype(np.float32), Bp.astype(np.float32),
                  Cp.astype(np.float32), Gam.astype(np.float32))


def _build_program():
    import concourse.tile as tile
    from concourse import bacc, mybir

    key = ("v18", WARM, F8PK, F8Y, DSCR)
    if key in _PROGRAM_CACHE:
        return _PROGRAM_CACHE[key]

    f32 = mybir.dt.float32
    bf16 = mybir.dt.bfloat16
    f8 = mybir.dt.float8e4
    xpk_dt = f8 if F8PK else bf16

    ndev = int(os.environ.get("KALMAN_NDEV", "1"))
    nc = bacc.Bacc("TRN2", target_bir_lowering=False, debug=False,
                   enable_asserts=False, num_devices=ndev,
                   dynamic_dma_scratch_size=DSCR)
    xb = nc.dram_tensor("xb", [128, CW], bf16, kind="ExternalInput").ap()
    xpk = nc.dram_tensor("xpk", [128, CW], xpk_dt, kind="ExternalInput").ap()
    yq_dt = f8 if F8Y else bf16
    yq = nc.dram_tensor("yq", [128, CW], yq_dt, kind="ExternalInput").ap()
    wb = nc.dram_tensor("wb", [128, 384], bf16, kind="ExternalInput").ap()
    if F8PK:
        w8 = nc.dram_tensor("w8", [128, 128], f8, kind="ExternalInput").ap()
    if F8Y:
        w8y = nc.dram_tensor("w8y", [128, 128], f8, kind="ExternalInput").ap()
    out = nc.dram_tensor("out", [128, NC], bf16, kind="ExternalOutput").ap()

    tiles = []
    c = 0
    while c < NC:
        tiles.append((c, min(TS, NC - c)))
        c += TS

    # per-tile input chunks; tile k touches cols [k*TS, (k+1)*TS+2), so the
    # first chunk is TS+2 wide and the rest shift by TS: tile k then depends
    # on chunks 0..k only.
    bounds = [0, TS + 2]
    while bounds[-1] + TS < CW:
        bounds.append(bounds[-1] + TS)
    bounds.append(CW)
    chunks = [(bounds[i], bounds[i + 1] - bounds[i])
              for i in range(len(bounds) - 1)]
    # the fp8 stream is half the bytes: pair up its chunks
    xpk_chunks = []
    for k in range(0, len(chunks), 2):
        pc0 = chunks[k][0]
        pcn = (chunks[k][1] + chunks[k + 1][1]
               if k + 1 < len(chunks) else chunks[k][1])
        xpk_chunks.append((pc0, pcn))

    with tile.TileContext(nc) as tc:
        with tc.tile_pool(name="consts", bufs=1) as consts, \
             tc.tile_pool(name="psw", bufs=1, space="PSUM") as psw_pool, \
             tc.tile_pool(name="ps", bufs=7, space="PSUM") as ps_pool:
            wbsb = consts.tile([128, 384], bf16)
            xsb = consts.tile([128, CW], bf16)
            xpsb = consts.tile([128, CW], xpk_dt)
            ysb = consts.tile([128, CW], yq_dt)
            osb = consts.tile([128, NC], bf16)
            if F8PK:
                w8sb = consts.tile([128, 128], f8)
                wosb = w8sb[:]
            else:
                wosb = wbsb[:, 128:256]
            if F8Y:
                w8ysb = consts.tile([128, 128], f8)
                wysb = w8ysb[:]
            else:
                wysb = wbsb[:, 256:384]
            scr = consts.tile([128, 512], bf16)

            # PE p-state warmup: matmuls over a memset scratch tile (no DMA
            # dependency) keep the tensor engine busy through the input load
            # so the clock has ramped when the real chain starts.
            nc.gpsimd.memset(scr[:], 0.0)
            if WARM:
                psw = psw_pool.tile([128, 512], f32)
                for _ in range(WARM):
                    nc.tensor.matmul(psw[:], scr[:, 0:128], scr[:],
                                     start=True, stop=True)

            # queue layout: sync = wb + x chunks, scalar = y chunks,
            # gpsimd (software DGE, slower) = the small fp8 stream
            nc.sync.dma_start(wbsb[:], wb[:])
            if F8PK:
                nc.gpsimd.dma_start(w8sb[:], w8[:])
            if F8Y:
                nc.gpsimd.dma_start(w8ysb[:], w8y[:])
            for (c0, cn) in chunks:
                nc.sync.dma_start(xsb[:, c0:c0 + cn], xb[:, c0:c0 + cn])
                nc.scalar.dma_start(ysb[:, c0:c0 + cn], yq[:, c0:c0 + cn])
            for (pc0, pcn) in xpk_chunks:
                nc.gpsimd.dma_start(xpsb[:, pc0:pc0 + pcn],
                                    xpk[:, pc0:pc0 + pcn])

            ndone = 0
            for ti, (c0, cn) in enumerate(tiles):
                ps = ps_pool.tile([128, cn], f32)
                # center x tap (sigma=0): moving offset c0+1
                nc.tensor.matmul(ps[:], wbsb[:, 0:128],
                                 xsb[:, c0 + 1:c0 + 1 + cn],
                                 start=True, stop=False)
                # both outer x taps in one pass against half-shifted x
                nc.tensor.matmul(ps[:], wosb,
                                 xpsb[:, c0:c0 + cn],
                                 start=False, stop=False)
                # all three y taps in one pass
                nc.tensor.matmul(ps[:], wysb,
                                 ysb[:, c0:c0 + cn],
                                 start=False, stop=True)
                nc.vector.tensor_copy(osb[:, c0:c0 + cn], ps[:])
                # drain finished output columns: after odd tiles and the
                # last three tiles, alternating scalar/gpsimd
                if ti % 2 == 1 or ti >= len(tiles) - 2:
                    o0, o1 = ndone, c0 + cn
                    ndone = o1
                    eng = nc.scalar if ti % 2 else nc.gpsimd
                    eng.dma_start(out[:, o0:o1], osb[:, o0:o1])
    nc.compile()
    _PROGRAM_CACHE[key] = nc
    return nc


def _fold(a, rows, width):
    # a: (rows, 16*width) -> (rows*16 partitions, width); partition b*rows+r
    # holds times t = c*16 + b
    return np.ascontiguousarray(
        a.reshape(rows, width, FOLD).transpose(2, 0, 1).reshape(
            FOLD * rows, width))


def _run_edge_strip(x0, y, Ap, Bp, Cp, Gam):
    x = x0.copy()
    for _ in range(ITERS):
        xp = np.concatenate([x[:, :1], x[:, :-1]], axis=1)
        xf_ = np.concatenate([x[:, 1:], x[:, -1:]], axis=1)
        x = (Ap @ x + Bp @ xp + Cp @ xf_ + Gam @ y).astype(np.float32)
    return x


def kernel(xs, ys, F, H, Q, R, gamma):
    import ml_dtypes
    from concourse.bass_utils import run_bass_kernel_spmd

    bf16 = np.dtype(ml_dtypes.bfloat16)

    xs = np.asarray(xs, dtype=np.float32)
    ysv = np.asarray(ys, dtype=np.float32)
    g = float(np.asarray(gamma))

    G, V, mats32 = _compose_taps(
        np.asarray(F, np.float64), np.asarray(H, np.float64),
        np.asarray(Q, np.float64), np.asarray(R, np.float64), g)
    K = ITERS
    D = DMAX
    # sanity: dropped taps must be tiny relative to the kept mass
    drop = max(np.abs(G[K + D + 1:K + 2 * D]).max(initial=0),
               np.abs(G[K - 2 * D:K - D]).max(initial=0))
    assert drop < 1e-4 * np.abs(G).max(), f"tap truncation too lossy: {drop}"

    # ---- block-banded weights, sigma in {-1,0,+1} == si in {0,1,2} ----
    WX = np.zeros((3, 128, 128), dtype=np.float32)
    WY = np.zeros((3, 64, 128), dtype=np.float32)
    for si in range(3):
        sig = si - 1
        for bo in range(FOLD):
            for bi in range(FOLD):
                d = sig * FOLD + bi - bo
                if abs(d) > D:
                    continue
                WX[si, bi * 8:bi * 8 + 8, bo * 8:bo * 8 + 8] = G[K + d].T
                WY[si, bi * 4:bi * 4 + 4, bo * 8:bo * 8 + 8] = V[K + d].T
    # D<=8 guarantees the outer blocks live in disjoint partition halves
    assert not WX[0][:64].any() and not WX[2][64:].any()
    assert not WY[0][:32].any() and not WY[2][32:].any()

    wb_np = np.zeros((128, 384), dtype=np.float32)
    wb_np[:, 0:128] = WX[1]
    # packed outer-x stationary: rows 0:64 pair with x shift +2 (sigma=+1),
    # rows 64:128 with x shift 0 (sigma=-1)
    wo_np = np.zeros((128, 128), dtype=np.float32)
    wo_np[0:64] = WX[2][:64]
    wo_np[64:128] = WX[0][64:]
    wb_np[:, 128:256] = wo_np
    # packed y stationary: rows 0:64 = center tap (y shift 1), rows 64:96 =
    # sigma=+1 block rows (y[0:32] shift 2), rows 96:128 = sigma=-1 block
    # rows (y[32:64] shift 0)
    wy_np = np.zeros((128, 128), dtype=np.float32)
    wy_np[0:64] = WY[1]
    wy_np[64:96] = WY[2][:32]
    wy_np[96:128] = WY[0][32:]
    wb_np[:, 256:384] = wy_np
    wb_np = wb_np.astype(bf16)

    # ---- per-core folded input windows ----
    pad = FOLD                               # S=1 halo in timesteps
    xw = FOLD * (CW + 2)
    xs_p = np.zeros((N, 7 * L + xw), dtype=np.float32)
    ys_p = np.zeros((M, 7 * L + xw), dtype=np.float32)
    xs_p[:, pad:pad + T] = xs
    ys_p[:, pad:pad + T] = ysv
    f8np = np.dtype(ml_dtypes.float8_e4m3)

    def to_f8(a):
        return a.astype(f8np)
    if F8PK:
        w8_np = to_f8(wo_np * F8S)
    if F8Y:
        w8y_np = to_f8(wy_np * F8S)
    in_maps = []
    for i in range(NCORES):
        o = i * L
        xf = _fold(xs_p[:, o:o + xw], N, CW + 2)
        yf = _fold(ys_p[:, o:o + xw], M, CW + 2)
        xpk_np = np.concatenate([xf[0:64, 2:CW + 2], xf[64:128, 0:CW]],
                                axis=0)
        yq_np = np.concatenate([yf[:, 1:CW + 1], yf[0:32, 2:CW + 2],
                                yf[32:64, 0:CW]], axis=0)
        m_ = {
            "xb": np.ascontiguousarray(xf[:, 0:CW]).astype(bf16),
            "xpk": to_f8(xpk_np / F8S) if F8PK else xpk_np.astype(bf16),
            "yq": to_f8(yq_np / F8S) if F8Y else yq_np.astype(bf16),
            "wb": wb_np,
        }
        if F8PK:
            m_["w8"] = w8_np
        if F8Y:
            m_["w8y"] = w8y_np
        in_maps.append(m_)

    nc = _build_program()
    trace = bool(int(os.environ.get("KALMAN_TRACE", "0")))
    res = run_bass_kernel_spmd(nc, in_maps, core_ids=list(range(NCORES)),
                               trace=trace)
    if trace and res.exec_time_ns is not None:
        print(f"HW exec time: {res.exec_time_ns} ns")
        print(f"HW exec time mean: {res.mean_exec_time_ns} ns")

    out_full = np.empty((N, T), dtype=np.float32)
    for i in range(NCORES):
        o = i * L
        Out = np.asarray(res.results[i]["out"]).astype(np.float32)  # (128,NC)
        unf = Out.reshape(FOLD, N, NC).transpose(1, 2, 0).reshape(N, FOLD * NC)
        out_full[:, o:o + L] = unf[:, :L]

    # ---- host edge strips (exact edge-replication dynamics) ----
    Ap32, Bp32, Cp32, Gam32 = mats32
    left = _run_edge_strip(xs[:, :STRIP], ysv[:, :STRIP],
                           Ap32, Bp32, Cp32, Gam32)
    right = _run_edge_strip(xs[:, -STRIP:], ysv[:, -STRIP:],
                            Ap32, Bp32, Cp32, Gam32)
    out_full[:, :EDGE] = left[:, :EDGE]
    out_full[:, -EDGE:] = right[:, -EDGE:]
    return out_full
